# revision 30
# baseline (speedup 1.0000x reference)
"""Trainium2 Bass kernel for nn_AdaptiveReasoningAmplifier.

Computation (B=1, S=8192, D=4096, f32):
  current    = mean(hidden_states, seq)                  # global -> needs AllReduce
  quality    = <current_hat, c*(||c||>0) - i*(||i||>0)>
  alpha      = piecewise(quality)
  m          = normalize(c - i)
  out[s,:]   = hidden[s,:] + clip(alpha * posw[s] * scale) * m

Sharding: sequence-parallel across 8 NeuronCores (1024 rows each, 16 MiB/core).
Each core keeps its shard resident in SBUF: one HBM read + one HBM write of
the big tensor; only a 16 KB partial-sum AllReduce crosses cores.

Device pipeline per core:
  1. DMA 8 tiles [128,4096] in; TensorE ones-matmul accumulates seq-sum
     into PSUM [1,4096] (overlapped with the loads).
  2. PSUM -> SBUF (DVE+ACT halves) -> DRAM -> AllReduce(16 KB) -> SBUF.
  3. Small math: ss=<S,S>, sd=<S,d> via DVE tensor_tensor_reduce + TensorE
     partition-sum; quality -> alpha; broadcast alpha; per-row coefficients
     cf[p,t] = min(alpha*pos_scale, 0.5).
  4. Per tile one fused DVE op: hs = (m_bcast * cf[p]) + hs; DMA out.
"""

import numpy as np

import concourse.bacc as bacc
import concourse.mybir as mybir
from concourse.tile import TileContext
from concourse.bass_utils import run_bass_kernel_spmd

N_CORES = 8
S = 8192
D = 4096
S_SH = S // N_CORES          # 1024 rows per core
P = 128                      # partitions
T = S_SH // P                # 8 tiles per core

MAX_STEERING = 0.5
AMP_THRESHOLD = 0.1
CORR_THRESHOLD = 0.3
EPS = 1e-12

_GRAPH = None


def build(s_total=S, s_sh=S_SH, d=D, warmup=True, use_ag=True):
    t_tiles = s_sh // P
    nb = (d + 511) // 512
    d32 = d // P

    nc = bacc.Bacc("TRN2", target_bir_lowering=False, num_devices=N_CORES)
    f32 = mybir.dt.float32

    hs = nc.declare_dram_parameter("hs", [s_sh, d], f32, isOutput=False)
    mvec = nc.declare_dram_parameter("mvec", [d], f32, isOutput=False)
    dvec = nc.declare_dram_parameter("dvec", [P, d // P], f32, isOutput=False)
    ps = nc.declare_dram_parameter("ps", [P, t_tiles], f32, isOutput=False)
    out = nc.declare_dram_parameter("out", [s_sh, d], f32, isOutput=True)

    with TileContext(nc) as tc:
        with (
            tc.tile_pool(name="hsp", bufs=t_tiles) as hsp,
            tc.tile_pool(name="aux", bufs=1) as aux,
            tc.tile_pool(name="psum", bufs=1, space="PSUM") as psump,
            tc.tile_pool(name="dram", bufs=1, space="DRAM") as dram,
        ):
            # constants / small inputs (off critical path)
            ones_col = aux.tile([P, 1], f32, tag="ones_col")
            nc.vector.memset(ones_col[:], 1.0)
            ones128 = aux.tile([P, P], f32, tag="ones128")
            nc.vector.memset(ones128[:], 1.0)
            ps_t = aux.tile([P, t_tiles], f32, tag="ps_t")
            nc.sync.dma_start(out=ps_t[:], in_=ps[:, :])
            dvec32 = aux.tile([P, d32], f32, tag="dvec32")
            nc.gpsimd.dma_start(out=dvec32[:], in_=dvec[:, :])
            m_bcast = aux.tile([P, d], f32, tag="m_bcast")
            nc.gpsimd.dma_start(out=m_bcast[0:1, :], in_=mvec[None, :])
            k = 1
            while k < P:
                nc.gpsimd.dma_start(
                    out=m_bcast[k : min(2 * k, P), :],
                    in_=m_bcast[0 : min(k, P - k), :],
                )
                k *= 2

            # warm-up collective: wakes ncfw + absorbs launch skew while
            # the big loads stream; makes the real collective warm.
            if warmup:
                wu_in = dram.tile([8], f32, tag="wu_in")
                wu_out = dram.tile([8 * N_CORES], f32, tag="wu_out")
                wu_sb = aux.tile([1, 8], f32, tag="wu_sb")
                nc.vector.memset(wu_sb[:], 0.0)
                nc.gpsimd.dma_start(out=wu_in[None, :], in_=wu_sb[:])
                if use_ag:
                    nc.gpsimd.collective_compute(
                        "AllGather",
                        mybir.AluOpType.bypass,
                        replica_groups=[list(range(N_CORES))],
                        ins=[wu_in.opt()],
                        outs=[wu_out.opt()],
                    )
                else:
                    nc.gpsimd.collective_compute(
                        "AllReduce",
                        mybir.AluOpType.add,
                        replica_groups=[list(range(N_CORES))],
                        ins=[wu_in.opt()],
                        outs=[wu_out[0 : 8].opt()],
                    )



            # phase 1: load shard + accumulate seq-sum on TensorE.
            # One full-PSUM tile reused for all PE outputs (seq-sum in
            # [0:1, :], stats in bank 0, alpha-bcast in bank 1 — the
            # later uses are WAR-ordered after the copies below).
            # DVE accumulator chain over tiles (hidden under the loads);
            # TensorE does the final partition-reduce of `acc` into
            # ps_full[0:1, :].  The last add + reduce matmuls + copies
            # are split in halves so they pipeline.
            ps_full = psump.tile([P, d], f32, tag="ps_full")
            acc = aux.tile([P, d], f32, tag="acc")
            sum_sb = aux.tile([1, d], f32, tag="sum_sb")
            hs_tiles = []
            last = t_tiles - 1
            half = d // 2
            for t in range(t_tiles):
                ht = hsp.tile([P, d], f32, tag="hs")
                nc.sync.dma_start(out=ht[:], in_=hs[t * P : (t + 1) * P, :])
                hs_tiles.append(ht)
                if t == 0:
                    continue
                in0 = hs_tiles[0] if t == 1 else acc
                if t == last:
                    nc.vector.tensor_add(
                        out=acc[:, 0:half], in0=in0[:, 0:half],
                        in1=ht[:, 0:half],
                    )
                    nc.vector.tensor_add(
                        out=acc[:, half:d], in0=in0[:, half:d],
                        in1=ht[:, half:d],
                    )
                else:
                    nc.vector.tensor_add(out=acc[:], in0=in0[:], in1=ht[:])
            for b in range(nb):
                nc.tensor.matmul(
                    ps_full[0:1, b * 512 : (b + 1) * 512],
                    ones_col[:, 0:1],
                    acc[:, b * 512 : (b + 1) * 512],
                    start=True,
                    stop=True,
                )
            # PSUM -> SBUF on two engines in parallel
            nc.vector.tensor_copy(
                out=sum_sb[0:1, 0:half], in_=ps_full[0:1, 0:half]
            )
            nc.scalar.copy(out=sum_sb[0:1, half:d], in_=ps_full[0:1, half:d])

            # phase 2: share the 16 KB partial sums.  AllGather + on-core
            # tree-sum beats AllReduce (1-hop mesh, no CCE reduce).
            sum_b = dram.tile([d], f32, tag="sum_b")
            nc.gpsimd.dma_start(out=sum_b[None, :], in_=sum_sb[:])
            s32 = aux.tile([P, d32], f32, tag="s32")
            if use_ag:
                ag_b = dram.tile([N_CORES * d], f32, tag="ag_b")
                nc.gpsimd.collective_compute(
                    "AllGather",
                    mybir.AluOpType.bypass,
                    replica_groups=[list(range(N_CORES))],
                    ins=[sum_b.opt()],
                    outs=[ag_b.opt()],
                )
                sall = aux.tile([P, N_CORES * d32], f32, tag="sall")
                nc.sync.dma_start(
                    out=sall.rearrange("p (r f) -> p r f", f=d32),
                    in_=ag_b.rearrange("(r p f) -> p r f", p=P, f=d32),
                )
                h4 = 4 * d32
                t1 = aux.tile([P, h4], f32, tag="t1")
                nc.vector.tensor_add(
                    out=t1[:], in0=sall[:, 0:h4], in1=sall[:, h4 : 2 * h4]
                )
                h2 = 2 * d32
                nc.vector.tensor_add(
                    out=t1[:, 0:h2], in0=t1[:, 0:h2], in1=t1[:, h2:h4]
                )
                nc.vector.tensor_add(
                    out=s32[:], in0=t1[:, 0:d32], in1=t1[:, d32:h2]
                )
            else:
                ar_b = dram.tile([d], f32, tag="ar_b")
                nc.gpsimd.collective_compute(
                    "AllReduce",
                    mybir.AluOpType.add,
                    replica_groups=[list(range(N_CORES))],
                    ins=[sum_b.opt()],
                    outs=[ar_b.opt()],
                )
                nc.sync.dma_start(
                    out=s32[:], in_=ar_b.rearrange("(p f) -> p f", p=P)
                )

            # phase 3: quality -> alpha -> cf.  The partition-sum matmul
            # uses an all-ones [P,P] stationary so [ss, sd] lands on
            # every partition at once; the whole scalar chain then runs
            # at [P,1] width and feeds cf without any re-broadcast.
            prod_ss = aux.tile([P, d32], f32, tag="prod_ss")
            prod_sd = aux.tile([P, d32], f32, tag="prod_sd")
            pp = aux.tile([P, 2], f32, tag="pp")
            nc.vector.scalar_tensor_tensor(
                out=prod_ss[:],
                in0=s32[:],
                scalar=1.0,
                in1=s32[:],
                op0=mybir.AluOpType.mult,
                op1=mybir.AluOpType.mult,
                accum_out=pp[:, 0:1],
            )
            nc.vector.scalar_tensor_tensor(
                out=prod_sd[:],
                in0=s32[:],
                scalar=1.0,
                in1=dvec32[:],
                op0=mybir.AluOpType.mult,
                op1=mybir.AluOpType.mult,
                accum_out=pp[:, 1:2],
            )
            nc.tensor.matmul(
                ps_full[0:P, 0:2], ones128[:, 0:P], pp[:, 0:2],
                start=True, stop=True,
            )

            sc = aux.tile([P, 6], f32, tag="sc")  # scratch scalars
            nrm = sc[:, 0:1]
            inv = sc[:, 1:2]
            q = sc[:, 2:3]
            amp = sc[:, 3:4]
            alpha = sc[:, 5:6]
            # nrm = max(sqrt(ss), S*eps)
            nc.scalar.sqrt(out=nrm, in_=ps_full[0:P, 0:1])
            nc.vector.tensor_scalar_max(
                out=nrm, in0=nrm, scalar1=float(s_total) * EPS
            )
            nc.vector.reciprocal(out=inv, in_=nrm)
            nc.vector.tensor_tensor(
                out=q, in0=ps_full[0:P, 1:2], in1=inv, op=mybir.AluOpType.mult
            )
            # amp = min(1.25*(0.1 - q), 0.5)
            nc.vector.tensor_scalar(
                out=amp,
                in0=q,
                scalar1=-(MAX_STEERING / (AMP_THRESHOLD + CORR_THRESHOLD)),
                scalar2=MAX_STEERING * AMP_THRESHOLD / (AMP_THRESHOLD + CORR_THRESHOLD),
                op0=mybir.AluOpType.mult,
                op1=mybir.AluOpType.add,
            )
            nc.vector.tensor_scalar_min(out=amp, in0=amp, scalar1=MAX_STEERING)
            # cond = (q < 0.1); alpha = (amp - 0.05)*cond + 0.05
            cond = sc[:, 4:5]
            nc.vector.tensor_scalar(
                out=cond, in0=q, scalar1=AMP_THRESHOLD, scalar2=None,
                op0=mybir.AluOpType.is_lt,
            )
            e2 = sc[:, 0:1]  # nrm slot is dead by now
            nc.vector.scalar_tensor_tensor(
                out=e2, in0=amp, scalar=-0.05, in1=cond,
                op0=mybir.AluOpType.add, op1=mybir.AluOpType.mult,
            )
            nc.vector.tensor_scalar_add(out=alpha, in0=e2, scalar1=0.05)

            cf = aux.tile([P, t_tiles], f32, tag="cf")
            nc.vector.tensor_scalar(
                out=cf[:],
                in0=ps_t[:],
                scalar1=alpha,
                scalar2=MAX_STEERING,
                op0=mybir.AluOpType.mult,
                op1=mybir.AluOpType.min,
            )

            # phase 4: fused steering add + store
            for t in range(t_tiles):
                ht = hs_tiles[t]
                eng = nc.vector
                eng.scalar_tensor_tensor(
                    out=ht[:],
                    in0=m_bcast[:],
                    scalar=cf[:, t : t + 1],
                    in1=ht[:],
                    op0=mybir.AluOpType.mult,
                    op1=mybir.AluOpType.add,
                )
                nc.sync.dma_start(out=out[t * P : (t + 1) * P, :], in_=ht[:])

    nc.compile()
    return nc


def _get_graph():
    global _GRAPH
    if _GRAPH is None:
        _GRAPH = build()
    return _GRAPH


def make_in_maps(hidden_states, correct_direction, incorrect_direction,
                 steering_scale, s_total=S, s_sh=S_SH, d=D):
    hs = np.ascontiguousarray(
        np.asarray(hidden_states, dtype=np.float32)[0]
    )  # [S, D]
    c = np.asarray(correct_direction, dtype=np.float32)
    i = np.asarray(incorrect_direction, dtype=np.float32)
    ssc = float(np.asarray(steering_scale).reshape(-1)[0])

    cn = np.linalg.norm(c)
    inn = np.linalg.norm(i)
    dvec = ((c if cn > 0 else 0.0 * c) - (i if inn > 0 else 0.0 * i)).astype(
        np.float32
    )
    # device computes dots on a [128, d/128] row-major view of the vectors
    dvec = np.ascontiguousarray(dvec.reshape(P, d // P))
    diff = c - i
    m = (diff / max(np.linalg.norm(diff), EPS)).astype(np.float32)

    rel_pos = np.arange(s_total, dtype=np.float32) / np.float32(s_total)
    pos_w = ((0.5 + 0.5 * rel_pos) * np.float32(ssc)).astype(np.float32)

    t_tiles = s_sh // P
    in_maps = []
    for cix in range(N_CORES):
        sh = np.ascontiguousarray(hs[cix * s_sh : (cix + 1) * s_sh])
        pw = pos_w[cix * s_sh : (cix + 1) * s_sh]
        in_maps.append(
            {
                "hs": sh,
                "mvec": m,
                "dvec": dvec,
                "ps": np.ascontiguousarray(pw.reshape(t_tiles, P).T),
            }
        )
    return in_maps


def kernel(hidden_states, correct_direction, incorrect_direction, steering_scale):
    nc = _get_graph()
    in_maps = make_in_maps(
        hidden_states, correct_direction, incorrect_direction, steering_scale
    )
    res = run_bass_kernel_spmd(nc, in_maps, core_ids=list(range(N_CORES)))
    full = np.concatenate(
        [res.results[i]["out"] for i in range(N_CORES)], axis=0
    )
    return full[None].astype(np.float32)


# revision 31
# speedup vs baseline: 1.0360x; 1.0360x over previous
"""Trainium2 Bass kernel for nn_AdaptiveReasoningAmplifier.

Computation (B=1, S=8192, D=4096, f32):
  current    = mean(hidden_states, seq)                  # global -> needs AllReduce
  quality    = <current_hat, c*(||c||>0) - i*(||i||>0)>
  alpha      = piecewise(quality)
  m          = normalize(c - i)
  out[s,:]   = hidden[s,:] + clip(alpha * posw[s] * scale) * m

Sharding: sequence-parallel across 8 NeuronCores (1024 rows each, 16 MiB/core).
Each core keeps its shard resident in SBUF: one HBM read + one HBM write of
the big tensor; only a 16 KB partial-sum AllReduce crosses cores.

Device pipeline per core:
  1. DMA 8 tiles [128,4096] in; TensorE ones-matmul accumulates seq-sum
     into PSUM [1,4096] (overlapped with the loads).
  2. PSUM -> SBUF (DVE+ACT halves) -> DRAM -> AllReduce(16 KB) -> SBUF.
  3. Small math: ss=<S,S>, sd=<S,d> via DVE tensor_tensor_reduce + TensorE
     partition-sum; quality -> alpha; broadcast alpha; per-row coefficients
     cf[p,t] = min(alpha*pos_scale, 0.5).
  4. Per tile one fused DVE op: hs = (m_bcast * cf[p]) + hs; DMA out.
"""

import numpy as np

import concourse.bacc as bacc
import concourse.mybir as mybir
from concourse.tile import TileContext
from concourse.bass_utils import run_bass_kernel_spmd

N_CORES = 8
S = 8192
D = 4096
S_SH = S // N_CORES          # 1024 rows per core
P = 128                      # partitions
T = S_SH // P                # 8 tiles per core

MAX_STEERING = 0.5
AMP_THRESHOLD = 0.1
CORR_THRESHOLD = 0.3
EPS = 1e-12

_GRAPH = None


def build(s_total=S, s_sh=S_SH, d=D, warmup=True, use_ag=True):
    t_tiles = s_sh // P
    nb = (d + 511) // 512
    d32 = d // P

    nc = bacc.Bacc("TRN2", target_bir_lowering=False, num_devices=N_CORES)
    f32 = mybir.dt.float32

    hs = nc.declare_dram_parameter("hs", [s_sh, d], f32, isOutput=False)
    mvec = nc.declare_dram_parameter("mvec", [d], f32, isOutput=False)
    dvec = nc.declare_dram_parameter("dvec", [P, d // P], f32, isOutput=False)
    ps = nc.declare_dram_parameter("ps", [P, t_tiles], f32, isOutput=False)
    out = nc.declare_dram_parameter("out", [s_sh, d], f32, isOutput=True)

    with TileContext(nc) as tc:
        with (
            tc.tile_pool(name="hsp", bufs=t_tiles) as hsp,
            tc.tile_pool(name="aux", bufs=1) as aux,
            tc.tile_pool(name="psum", bufs=1, space="PSUM") as psump,
            tc.tile_pool(name="dram", bufs=1, space="DRAM") as dram,
        ):
            # constants / small inputs (off critical path)
            ones_col = aux.tile([P, 1], f32, tag="ones_col")
            nc.vector.memset(ones_col[:], 1.0)
            ones128 = aux.tile([P, P], f32, tag="ones128")
            nc.vector.memset(ones128[:], 1.0)
            ps_t = aux.tile([P, t_tiles], f32, tag="ps_t")
            nc.sync.dma_start(out=ps_t[:], in_=ps[:, :])
            dvec32 = aux.tile([P, d32], f32, tag="dvec32")
            nc.gpsimd.dma_start(out=dvec32[:], in_=dvec[:, :])
            m_bcast = aux.tile([P, d], f32, tag="m_bcast")
            nc.gpsimd.dma_start(out=m_bcast[0:1, :], in_=mvec[None, :])
            k = 1
            while k < P:
                nc.gpsimd.dma_start(
                    out=m_bcast[k : min(2 * k, P), :],
                    in_=m_bcast[0 : min(k, P - k), :],
                )
                k *= 2

            # warm-up collective: wakes ncfw + absorbs launch skew while
            # the big loads stream; makes the real collective warm.
            if warmup:
                wu_in = dram.tile([8], f32, tag="wu_in")
                wu_out = dram.tile([8 * N_CORES], f32, tag="wu_out")
                wu_sb = aux.tile([1, 8], f32, tag="wu_sb")
                nc.vector.memset(wu_sb[:], 0.0)
                nc.gpsimd.dma_start(out=wu_in[None, :], in_=wu_sb[:])
                if use_ag:
                    nc.gpsimd.collective_compute(
                        "AllGather",
                        mybir.AluOpType.bypass,
                        replica_groups=[list(range(N_CORES))],
                        ins=[wu_in.opt()],
                        outs=[wu_out.opt()],
                    )
                else:
                    nc.gpsimd.collective_compute(
                        "AllReduce",
                        mybir.AluOpType.add,
                        replica_groups=[list(range(N_CORES))],
                        ins=[wu_in.opt()],
                        outs=[wu_out[0 : 8].opt()],
                    )



            # phase 1: load shard + accumulate seq-sum on TensorE.
            # One full-PSUM tile reused for all PE outputs (seq-sum in
            # [0:1, :], stats in bank 0, alpha-bcast in bank 1 — the
            # later uses are WAR-ordered after the copies below).
            # DVE accumulator chain over tiles (hidden under the loads);
            # TensorE does the final partition-reduce of `acc` into
            # ps_full[0:1, :].  The last add + reduce matmuls + copies
            # are split in halves so they pipeline.
            ps_full = psump.tile([P, d], f32, tag="ps_full")
            acc = aux.tile([P, d], f32, tag="acc")
            sum_sb = aux.tile([1, d], f32, tag="sum_sb")
            hs_tiles = []
            last = t_tiles - 1
            half = d // 2
            for t in range(t_tiles):
                ht = hsp.tile([P, d], f32, tag="hs")
                nc.sync.dma_start(out=ht[:], in_=hs[t * P : (t + 1) * P, :])
                hs_tiles.append(ht)
                if t == 0:
                    continue
                in0 = hs_tiles[0] if t == 1 else acc
                if t == last:
                    nc.vector.tensor_add(
                        out=acc[:, 0:half], in0=in0[:, 0:half],
                        in1=ht[:, 0:half],
                    )
                    nc.vector.tensor_add(
                        out=acc[:, half:d], in0=in0[:, half:d],
                        in1=ht[:, half:d],
                    )
                else:
                    nc.vector.tensor_add(out=acc[:], in0=in0[:], in1=ht[:])
            for b in range(nb):
                nc.tensor.matmul(
                    ps_full[0:1, b * 512 : (b + 1) * 512],
                    ones_col[:, 0:1],
                    acc[:, b * 512 : (b + 1) * 512],
                    start=True,
                    stop=True,
                )
            # PSUM -> SBUF on two engines in parallel
            nc.vector.tensor_copy(
                out=sum_sb[0:1, 0:half], in_=ps_full[0:1, 0:half]
            )
            nc.scalar.copy(out=sum_sb[0:1, half:d], in_=ps_full[0:1, half:d])

            # phase 2: share the 16 KB partial sums.  AllGather + on-core
            # tree-sum beats AllReduce (1-hop mesh, no CCE reduce).
            sum_b = dram.tile([d], f32, tag="sum_b")
            nc.gpsimd.dma_start(out=sum_b[None, :], in_=sum_sb[:])
            s32 = aux.tile([P, d32], f32, tag="s32")
            if use_ag:
                ag_b = dram.tile([N_CORES * d], f32, tag="ag_b")
                nc.gpsimd.collective_compute(
                    "AllGather",
                    mybir.AluOpType.bypass,
                    replica_groups=[list(range(N_CORES))],
                    ins=[sum_b.opt()],
                    outs=[ag_b.opt()],
                )
                sall = aux.tile([P, N_CORES * d32], f32, tag="sall")
                nc.sync.dma_start(
                    out=sall.rearrange("p (r f) -> p r f", f=d32),
                    in_=ag_b.rearrange("(r p f) -> p r f", p=P, f=d32),
                )
                h4 = 4 * d32
                t1 = aux.tile([P, h4], f32, tag="t1")
                nc.vector.tensor_add(
                    out=t1[:], in0=sall[:, 0:h4], in1=sall[:, h4 : 2 * h4]
                )
                h2 = 2 * d32
                nc.vector.tensor_add(
                    out=t1[:, 0:h2], in0=t1[:, 0:h2], in1=t1[:, h2:h4]
                )
                nc.vector.tensor_add(
                    out=s32[:], in0=t1[:, 0:d32], in1=t1[:, d32:h2]
                )
            else:
                ar_b = dram.tile([d], f32, tag="ar_b")
                nc.gpsimd.collective_compute(
                    "AllReduce",
                    mybir.AluOpType.add,
                    replica_groups=[list(range(N_CORES))],
                    ins=[sum_b.opt()],
                    outs=[ar_b.opt()],
                )
                nc.sync.dma_start(
                    out=s32[:], in_=ar_b.rearrange("(p f) -> p f", p=P)
                )

            # phase 3: quality -> alpha -> cf.  The partition-sum matmul
            # uses an all-ones [P,P] stationary so [ss, sd] lands on
            # every partition at once; the whole scalar chain then runs
            # at [P,1] width and feeds cf without any re-broadcast.
            prod_ss = aux.tile([P, d32], f32, tag="prod_ss")
            prod_sd = aux.tile([P, d32], f32, tag="prod_sd")
            pp = aux.tile([P, 2], f32, tag="pp")
            nc.vector.scalar_tensor_tensor(
                out=prod_ss[:],
                in0=s32[:],
                scalar=1.0,
                in1=s32[:],
                op0=mybir.AluOpType.mult,
                op1=mybir.AluOpType.mult,
                accum_out=pp[:, 0:1],
            )
            nc.vector.scalar_tensor_tensor(
                out=prod_sd[:],
                in0=s32[:],
                scalar=1.0,
                in1=dvec32[:],
                op0=mybir.AluOpType.mult,
                op1=mybir.AluOpType.mult,
                accum_out=pp[:, 1:2],
            )
            nc.tensor.matmul(
                ps_full[0:P, 0:2], ones128[:, 0:P], pp[:, 0:2],
                start=True, stop=True,
            )

            sc = aux.tile([P, 6], f32, tag="sc")  # scratch scalars
            nrm = sc[:, 0:1]
            inv = sc[:, 1:2]
            q = sc[:, 2:3]
            amp = sc[:, 3:4]
            alpha = sc[:, 5:6]
            # nrm = max(sqrt(ss), S*eps)
            nc.scalar.sqrt(out=nrm, in_=ps_full[0:P, 0:1])
            nc.vector.tensor_scalar_max(
                out=nrm, in0=nrm, scalar1=float(s_total) * EPS
            )
            nc.vector.reciprocal(out=inv, in_=nrm)
            nc.vector.tensor_tensor(
                out=q, in0=ps_full[0:P, 1:2], in1=inv, op=mybir.AluOpType.mult
            )
            # amp = min(1.25*(0.1 - q), 0.5)
            nc.vector.tensor_scalar(
                out=amp,
                in0=q,
                scalar1=-(MAX_STEERING / (AMP_THRESHOLD + CORR_THRESHOLD)),
                scalar2=MAX_STEERING * AMP_THRESHOLD / (AMP_THRESHOLD + CORR_THRESHOLD),
                op0=mybir.AluOpType.mult,
                op1=mybir.AluOpType.add,
            )
            nc.vector.tensor_scalar_min(out=amp, in0=amp, scalar1=MAX_STEERING)
            # cond = (q < 0.1); alpha = (amp - 0.05)*cond + 0.05
            cond = sc[:, 4:5]
            nc.vector.tensor_scalar(
                out=cond, in0=q, scalar1=AMP_THRESHOLD, scalar2=None,
                op0=mybir.AluOpType.is_lt,
            )
            e2 = sc[:, 0:1]  # nrm slot is dead by now
            nc.vector.scalar_tensor_tensor(
                out=e2, in0=amp, scalar=-0.05, in1=cond,
                op0=mybir.AluOpType.add, op1=mybir.AluOpType.mult,
            )
            nc.vector.tensor_scalar_add(out=alpha, in0=e2, scalar1=0.05)

            cf = aux.tile([P, t_tiles], f32, tag="cf")
            nc.vector.tensor_scalar(
                out=cf[:],
                in0=ps_t[:],
                scalar1=alpha,
                scalar2=MAX_STEERING,
                op0=mybir.AluOpType.mult,
                op1=mybir.AluOpType.min,
            )

            # phase 4: fused steering add + store
            for t in range(t_tiles):
                ht = hs_tiles[t]
                eng = nc.vector
                eng.scalar_tensor_tensor(
                    out=ht[:],
                    in0=m_bcast[:],
                    scalar=cf[:, t : t + 1],
                    in1=ht[:],
                    op0=mybir.AluOpType.mult,
                    op1=mybir.AluOpType.add,
                )
                nc.sync.dma_start(out=out[t * P : (t + 1) * P, :], in_=ht[:])

    nc.compile()
    return nc


def _get_graph():
    global _GRAPH
    if _GRAPH is None:
        import os

        _GRAPH = build(
            warmup=os.environ.get("K_WARMUP", "1") == "1",
            use_ag=os.environ.get("K_AG", "1") == "1",
        )
    return _GRAPH


def make_in_maps(hidden_states, correct_direction, incorrect_direction,
                 steering_scale, s_total=S, s_sh=S_SH, d=D):
    hs = np.ascontiguousarray(
        np.asarray(hidden_states, dtype=np.float32)[0]
    )  # [S, D]
    c = np.asarray(correct_direction, dtype=np.float32)
    i = np.asarray(incorrect_direction, dtype=np.float32)
    ssc = float(np.asarray(steering_scale).reshape(-1)[0])

    cn = np.linalg.norm(c)
    inn = np.linalg.norm(i)
    dvec = ((c if cn > 0 else 0.0 * c) - (i if inn > 0 else 0.0 * i)).astype(
        np.float32
    )
    # device computes dots on a [128, d/128] row-major view of the vectors
    dvec = np.ascontiguousarray(dvec.reshape(P, d // P))
    diff = c - i
    m = (diff / max(np.linalg.norm(diff), EPS)).astype(np.float32)

    rel_pos = np.arange(s_total, dtype=np.float32) / np.float32(s_total)
    pos_w = ((0.5 + 0.5 * rel_pos) * np.float32(ssc)).astype(np.float32)

    t_tiles = s_sh // P
    in_maps = []
    for cix in range(N_CORES):
        sh = np.ascontiguousarray(hs[cix * s_sh : (cix + 1) * s_sh])
        pw = pos_w[cix * s_sh : (cix + 1) * s_sh]
        in_maps.append(
            {
                "hs": sh,
                "mvec": m,
                "dvec": dvec,
                "ps": np.ascontiguousarray(pw.reshape(t_tiles, P).T),
            }
        )
    return in_maps


def kernel(hidden_states, correct_direction, incorrect_direction, steering_scale):
    nc = _get_graph()
    in_maps = make_in_maps(
        hidden_states, correct_direction, incorrect_direction, steering_scale
    )
    res = run_bass_kernel_spmd(nc, in_maps, core_ids=list(range(N_CORES)))
    full = np.concatenate(
        [res.results[i]["out"] for i in range(N_CORES)], axis=0
    )
    return full[None].astype(np.float32)
